# revision 72
# baseline (speedup 1.0000x reference)
"""Multi-head causal self-attention with RoPE on 8 Trainium2 NeuronCores.

Problem: x[2,2048,2048], wq/wk/wv/wo[2048,2048] fp32, 16 heads (hd=128),
interleaved RoPE, causal softmax.

Sharding: (batch, head-group) parallel — each core owns ONE batch and FOUR
heads (cores 0-3 -> b=0 head-groups 0-3, cores 4-7 -> b=1).  wo is
row-sharded; host sums the 4 partial y's per batch.

All data bf16 (1 col/cycle PE rate, half DMA/SBUF); PSUM stays fp32.

Engine assignment (balanced so the PE never waits):
  PE     : projections, RoPE pair-swap, scores, attnV, per-pair ones-matmul
           (softmax denominator), output projection
  ACT    : exp; PSUM->SBUF copies during projection sections (idle there)
  DVE    : RoPE cos/sin, causal mask, rowsum accumulate, reciprocal,
           normalize, half the y copies
  GpSimd : x DMA issue + odd weight tiles at startup

Fused pipeline per core, per s-chunk sc (512 cols):
  - project chunk sc (fine-grained per-(e,chunk) tiles)
  - attention q-block j=sc: heads in 2 pairs; per kv tile t: scores for
    both heads into one 2-bank PSUM tile -> single exp (3D AP over both
    heads' valid staircase regions) -> DVE mask -> DVE-accumulate into a
    per-pair fp32 rowsum accumulator; attnV matmuls run 2 tiles behind
    (pend queue crossing pair boundaries; masked staircase bands split off
    and deferred one extra drain so the PE never waits on the mask).  The
    rowsum accumulator is bf16 (single ~0.4% denominator rounding, engages
    the fast DVE path); at each pair's last tile one ones-matmul per head
    forms the denominator and the reciprocal is ready before the last
    attnV drains, so the pair's PSUM frees ~1us after its last matmul.
  - output projection for block j is DEFERRED: emitted as closures popped
    inside block j+1's t-loop, filling PE slots that the ACT-bound exp
    stream would otherwise leave idle; y copies alternate DVE/ACT.

DMA: weights split by (d-tile, q|kv column group) across both rings in
consumption order so the first projection chain is never starved; the
next chunk's x prefetches during attention; y leaves as 256-row slabs on
the sync ring.
"""

import os
import sys

for _p in ("/opt/trn_rl_repo", "/root/.axon_site/_ro/trn_rl_repo"):
    if os.path.isdir(_p) and _p not in sys.path:
        sys.path.append(_p)

import numpy as np

import concourse.bacc as bacc
import concourse.bass_isa as bass_isa
import concourse.mybir as mybir
import concourse.tile as tile
from concourse.alu_op_type import AluOpType
from concourse.bass_utils import run_bass_kernel_spmd

F32 = mybir.dt.float32
F32R = mybir.dt.float32r
BF16 = mybir.dt.bfloat16

B, S, D = 2, 2048, 2048
H, HD = 16, 128
NCORES = 8
HPC = 4                      # heads per core
CPC = HPC * HD               # channels per core = 512
P = 128
SC = 512                     # s-chunk for projections / q-block for attention
NSC = S // SC                # 4
NDT = D // P                 # 16 contraction tiles
NG = 4                       # x-tile DMA group: d-tiles per DMA
QC = 4 * P                   # q-head weight columns (512)
KVC = 8 * P                  # k-head + v weight columns (1024)
WCOLS = QC + KVC             # 1536
ROPE_THETA = 10000.0

Exp = mybir.ActivationFunctionType.Exp

last_exec_time_ns = None
_nc_cache = None


def _build_nc():
    nc = bacc.Bacc("TRN2", target_bir_lowering=False, debug=False)

    xT = nc.dram_tensor("xT", [D, S], BF16, kind="ExternalInput")
    wqT = nc.dram_tensor("wqT", [D, QC], BF16, kind="ExternalInput")
    wkvT = nc.dram_tensor("wkvT", [D, KVC], BF16, kind="ExternalInput")
    woT = nc.dram_tensor("woT", [CPC, D], BF16, kind="ExternalInput")
    cosT = nc.dram_tensor("cosT", [HD, S], BF16, kind="ExternalInput")
    sinT = nc.dram_tensor("sinT", [HD, S], BF16, kind="ExternalInput")
    rotL = nc.dram_tensor("rotL", [HD, HD], BF16, kind="ExternalInput")
    trimask = nc.dram_tensor("trimask", [P, 2 * P], BF16, kind="ExternalInput")
    ones = nc.dram_tensor("ones", [P, P], BF16, kind="ExternalInput")
    yT = nc.dram_tensor("yT", [D, S], BF16, kind="ExternalOutput")

    xTr = xT.rearrange("(o p) s -> p o s", p=P)
    wqrQ = wqT.rearrange("(o p) e -> p o e", p=P)
    wqrKV = wkvT.rearrange("(o p) e -> p o e", p=P)

    with tile.TileContext(nc) as tc:
        with tc.tile_pool(name="const", bufs=1) as constp, \
             tc.tile_pool(name="xp", bufs=8) as xp, \
             tc.tile_pool(name="qk", bufs=1) as qkp, \
             tc.tile_pool(name="vp", bufs=1) as vp, \
             tc.tile_pool(name="op", bufs=2) as op_, \
             tc.tile_pool(name="attn", bufs=7) as attnp, \
             tc.tile_pool(name="acs", bufs=2) as accsp, \
             tc.tile_pool(name="acb", bufs=2) as accbp, \
             tc.tile_pool(name="tmp", bufs=2) as tmpp, \
             tc.tile_pool(name="rc", bufs=4) as rcp, \
             tc.tile_pool(name="yt", bufs=4) as ytp, \
             tc.tile_pool(name="ps", bufs=2, space="PSUM") as psp, \
             tc.tile_pool(name="po", bufs=1, space="PSUM") as pop, \
             tc.tile_pool(name="acc", bufs=2, space="PSUM") as accp:

            # ---- weights & first x chunk, both rings, consumption order:
            #      (q-cols + x chunk 0) first, then kv-cols, then the rest ----
            wqQ_t = [constp.tile([P, QC], BF16, name=f"wqQ{dt}")
                     for dt in range(NDT)]
            wqKV_t = [constp.tile([P, KVC], BF16, name=f"wqKV{dt}")
                      for dt in range(NDT)]
            # both rings in parallel from t=0: gpsimd carries the first x
            # chunk + odd weight tiles, sync the even weight tiles + consts
            xg0 = [xp.tile([P, NG, SC], BF16, tag="xt", name=f"xt0_{g}")
                   for g in range(NDT // NG)]
            nc.gpsimd.dma_start(xg0[0][:], xTr[:, 0:NG, 0:SC])
            nc.sync.dma_start(wqQ_t[0][:], wqrQ[:, 0, :])
            for dt in range(1, NDT, 2):
                nc.gpsimd.dma_start(wqQ_t[dt][:], wqrQ[:, dt, :])
            for dt in range(2, NDT, 2):
                nc.sync.dma_start(wqQ_t[dt][:], wqrQ[:, dt, :])
            for g in range(1, NDT // NG):
                nc.gpsimd.dma_start(xg0[g][:],
                                    xTr[:, g * NG:(g + 1) * NG, 0:SC])
            rot_sb = constp.tile([P, P], BF16)
            cos_sb = constp.tile([P, S], BF16)
            sin_sb = constp.tile([P, S], BF16)
            mask_sb = constp.tile([P, 2, P], BF16)
            nc.sync.dma_start(rot_sb[:], rotL[:])
            nc.sync.dma_start(cos_sb[:], cosT[:])
            nc.sync.dma_start(sin_sb[:], sinT[:])
            ones_sb = constp.tile([P, P], BF16)
            nc.sync.dma_start(
                mask_sb[:], trimask.rearrange("p (n q) -> p n q", n=2))
            nc.sync.dma_start(ones_sb[:], ones[:])
            for dt in range(1, NDT, 2):
                nc.gpsimd.dma_start(wqKV_t[dt][:], wqrKV[:, dt, :])
            for dt in range(0, NDT, 2):
                nc.sync.dma_start(wqKV_t[dt][:], wqrKV[:, dt, :])
            wo_sb = constp.tile([P, HPC, D], BF16)
            nc.sync.dma_start(wo_sb[:], woT.rearrange("(o p) e -> p o e", p=P))

            qkc = [[qkp.tile([P, SC], BF16, tag=f"qk{e}_{c}", name=f"qk{e}_{c}")
                    for c in range(NSC)] for e in range(8)]
            v_c = [vp.tile([P, SC // P, CPC], BF16, tag=f"v{c}", name=f"v{c}")
                   for c in range(NSC)]

            def wcol(dt, e):  # weight slice for channel-tile e of d-tile dt
                if e < 4:
                    return wqQ_t[dt][:, e * P:(e + 1) * P]
                return wqKV_t[dt][:, (e - 4) * P:(e - 3) * P]

            def proj_chunk(sc, xts, ng):
                sl = slice(sc * SC, (sc + 1) * SC)

                def mk_rope(e):
                    # RoPE for channel-tile e, deferred one e-tile so the
                    # rotation matmul never stalls behind the ACT copy
                    def emit():
                        q = qkc[e][sc]
                        pr = accp.tile([P, SC], F32, tag="acc",
                                       name=f"pr{e}_{sc}")
                        nc.tensor.matmul(pr[:], rot_sb[:], q[:],
                                         start=True, stop=True)
                        tmp = tmpp.tile([P, SC], BF16, tag="ropetmp")
                        nc.vector.tensor_tensor(tmp[:], pr[:], sin_sb[:, sl],
                                                AluOpType.mult)
                        nc.vector.tensor_tensor(q[:], q[:], cos_sb[:, sl],
                                                AluOpType.mult)
                        nc.vector.tensor_tensor(q[:], q[:], tmp[:],
                                                AluOpType.add)
                    return emit

                def proj_psum(k, name):
                    # alternate accumulators between the "acc" pool and the
                    # (projection-idle) score pool so a chain's start never
                    # waits on the previous chain's ACT evacuation
                    if k % 2 == 0:
                        return accp.tile([P, SC], F32, tag="acc", name=name)[:]
                    return psp.tile([P, 2, SC], F32, tag="ps",
                                    name=name)[:, 0, :]

                rope_pend = None
                for e in range(8):
                    pq = proj_psum(e, f"pq{e}_{sc}")
                    for dt in range(NDT):
                        nc.tensor.matmul(pq, wcol(dt, e),
                                         xts[dt // ng][:, dt % ng, :],
                                         start=(dt == 0), stop=(dt == NDT - 1))
                    nc.scalar.copy(qkc[e][sc][:], pq)
                    if rope_pend is not None:
                        rope_pend()
                    rope_pend = mk_rope(e)
                for ss in range(SC // P):
                    if ss == 1 and rope_pend is not None:
                        rope_pend()
                        rope_pend = None
                    pv = proj_psum(ss, f"pv{ss}_{sc}")
                    for dt in range(NDT):
                        nc.tensor.matmul(pv,
                                         xts[dt // ng][:, dt % ng,
                                                       ss * P:(ss + 1) * P],
                                         wqKV_t[dt][:, 4 * P:],
                                         start=(dt == 0), stop=(dt == NDT - 1))
                    nc.scalar.copy(v_c[sc][:, ss, :], pv)

            def attn_block(j, deferred):
                """Attention q-block j; pops `deferred` closures (previous
                block's gpsimd-pair tail + output projection) into PE idle
                slots.  Returns this block's closures."""
                n_kv = (SC // P) * (j + 1)
                oT = op_.tile([P, HPC, SC], BF16, tag="o", name=f"oT{j}")
                pend = []

                band_pend = []

                def emit_band():
                    # the masked 128-col band of a staircase tile, deferred
                    # one extra drain so the PE never waits on the DVE mask
                    st, attn, t, dlt, glast = band_pend.pop(0)
                    for i in range(2):
                        nc.tensor.matmul(st["po"][:, i, dlt:dlt + P],
                                         v_c[t // 4][:, t % 4,
                                                     st["h"][i] * HD:
                                                     (st["h"][i] + 1) * HD],
                                         attn[:, i, dlt:dlt + P],
                                         start=False, stop=glast,
                                         skip_group_check=True)
                    if glast:  # group complete: normalize (recip is ready)
                        for i in range(2):
                            nc.vector.tensor_tensor(oT[:, st["h"][i], :],
                                                    st["po"][:, i, :],
                                                    st["recip"][i][:],
                                                    AluOpType.mult)

                def drain_one():
                    st, attn, t, dlt, dp = pend.pop(0)
                    first = (t == 0)
                    if dp < 0:  # full tile: single mask-free matmul
                        for i in range(2):
                            nc.tensor.matmul(st["po"][:, i, :],
                                             v_c[t // 4][:, t % 4,
                                                         st["h"][i] * HD:
                                                         (st["h"][i] + 1) * HD],
                                             attn[:, i, :],
                                             start=first, stop=False,
                                             skip_group_check=True)
                    else:
                        if dlt + P < SC:  # mask-independent tail now
                            for i in range(2):
                                nc.tensor.matmul(st["po"][:, i, dlt + P:],
                                                 v_c[t // 4][:, t % 4,
                                                             st["h"][i] * HD:
                                                             (st["h"][i] + 1)
                                                             * HD],
                                                 attn[:, i, dlt + P:],
                                                 start=first, stop=False,
                                                 skip_group_check=True)
                        band_pend.append((st, attn, t, dlt, t == n_kv - 1))
                        if len(band_pend) > 1:
                            emit_band()

                it = 0
                niters = 2 * n_kv
                for pair in range(2):
                    hA, hB = 2 * pair, 2 * pair + 1
                    st = {
                        "h": (hA, hB),
                        "po": pop.tile([P, 2, SC], F32, tag="po",
                                       name=f"po{j}_{pair}"),
                        "acc": accsp.tile([P, 2, SC], BF16, tag="acs",
                                          name=f"acs{j}_{pair}"),
                    }
                    for t in range(n_kv):
                        dp = t - (SC // P) * j
                        dlt = max(dp, 0) * P  # first valid column
                        psc = psp.tile([P, 2, SC], F32, tag="ps",
                                       name=f"psc{j}_{pair}_{t}")
                        for i, h in enumerate((hA, hB)):
                            nc.tensor.matmul(psc[:, i, dlt:],
                                             qkc[4 + h][t // 4][:, (t % 4) * P:
                                                                (t % 4 + 1) * P],
                                             qkc[h][j][:, dlt:],
                                             start=True, stop=True)
                        attn = attnp.tile([P, 2, SC], BF16, tag="attn")
                        nc.scalar.activation(attn[:, :, dlt:], psc[:, :, dlt:],
                                             Exp, bias=0.0, scale=1.0)
                        if dp >= 0:  # triangle mask on the 128-col band
                            nc.vector.tensor_tensor(
                                attn[:, :, dlt:dlt + P],
                                attn[:, :, dlt:dlt + P],
                                mask_sb[:], AluOpType.mult)
                        if t == 0:  # rowsum accumulator init / accumulate
                            nc.vector.tensor_scalar_mul(st["acc"][:],
                                                        attn[:], 1.0)
                        else:
                            nc.vector.tensor_tensor(st["acc"][:, :, dlt:],
                                                    st["acc"][:, :, dlt:],
                                                    attn[:, :, dlt:],
                                                    AluOpType.add)
                        if t == n_kv - 1:
                            # denominator + reciprocal as soon as the rowsum
                            # is complete; the drain only normalizes.  The
                            # accumulator is bf16 (one ~0.4% rounding on the
                            # denominator, well inside tolerance) so the
                            # ones-matmul reads it with no conversion pass
                            st["recip"] = []
                            for i in range(2):
                                prs = accp.tile([P, SC], F32, tag="acc",
                                                name=f"prs{j}_{st['h'][i]}")
                                nc.tensor.matmul(prs[:], ones_sb[:],
                                                 st["acc"][:, i, :],
                                                 start=True, stop=True)
                                recip = rcp.tile([P, SC], F32, tag="recip")
                                nc.vector.reciprocal_approx_fast(recip[:],
                                                                 prs[:])
                                st["recip"].append(recip)
                        pend.append((st, attn, t, dlt, dp))
                        if len(pend) > 4:
                            drain_one()
                        # pop deferred work: the previous block's gpsimd-pair
                        # tail first (chain already done), its output
                        # projection from mid-loop
                        if deferred and (it == 0 or it >= niters // 2):
                            deferred.pop(0)()
                        it += 1
                while pend:
                    drain_one()
                while band_pend:
                    emit_band()
                while deferred:
                    deferred.pop(0)()

                jsl = slice(j * SC, (j + 1) * SC)

                def mk_closure(e2):
                    def emit():
                        yt = ytp.tile([P, 2, SC], BF16, tag="yt")
                        for si in range(2):
                            et = e2 * 2 + si
                            py = accp.tile([P, SC], F32, tag="acc",
                                           name=f"py{j}_{et}")
                            for ct in range(HPC):
                                nc.tensor.matmul(
                                    py[:],
                                    wo_sb[:, ct, et * P:(et + 1) * P],
                                    oT[:, ct, :],
                                    start=(ct == 0), stop=(ct == HPC - 1))
                            if si == 0:  # split copies across DVE and ACT
                                nc.vector.tensor_scalar_mul(yt[:, si, :],
                                                            py[:], 1.0)
                            else:
                                nc.scalar.copy(yt[:, si, :], py[:])
                        nc.sync.dma_start(
                            yT[e2 * 2 * P:(e2 + 1) * 2 * P, jsl]
                            .rearrange("(n p) q -> p n q", p=P),
                            yt[:])
                    return emit

                return [mk_closure(e2) for e2 in range(8)]

            deferred = []
            xts_next = xg0
            for sc in range(NSC):
                proj_chunk(sc, xts_next, NG)
                if sc + 1 < NSC:  # prefetch next chunk's x during attention
                    xts_next = []
                    for g in range(NDT // NG):
                        xt = xp.tile([P, NG, SC], BF16, tag="xt")
                        nc.gpsimd.dma_start(
                            xt[:], xTr[:, g * NG:(g + 1) * NG,
                                       (sc + 1) * SC:(sc + 2) * SC])
                        xts_next.append(xt)
                deferred = attn_block(sc, deferred)
            for cl in deferred:  # final block's output projection
                cl()
    nc.finalize()
    return nc


def _host_inputs(x, wq, wk, wv, wo):
    """Build per-core input maps (host-side shard + transform)."""
    import ml_dtypes
    bf16 = ml_dtypes.bfloat16
    scale = 1.0 / np.sqrt(np.float32(HD))

    # RoPE tables in [e, s] layout (same for every head)
    inv_freq = 1.0 / (ROPE_THETA ** (np.arange(0, HD, 2, dtype=np.float64) / HD))
    ang = np.arange(S, dtype=np.float64)[None, :] * inv_freq[:, None]  # [64, S]
    cosT = np.repeat(np.cos(ang), 2, axis=0).astype(bf16)  # [128, S]
    sinT = np.repeat(np.sin(ang), 2, axis=0).astype(bf16)

    # signed pair-swap: qrot[2i] = -q[2i+1], qrot[2i+1] = q[2i]
    # matmul computes qrot[m, s] = sum_k rotL[k, m] q[k, s]
    rotL = np.zeros((HD, HD), dtype=np.float32)
    for i in range(HD // 2):
        rotL[2 * i + 1, 2 * i] = -1.0
        rotL[2 * i, 2 * i + 1] = 1.0
    rotL = rotL.astype(bf16)

    r = np.arange(P)[:, None]
    c = np.arange(P)[None, :]
    tri = (c >= r).astype(bf16)  # [128,128] upper-right valid
    trimask = np.concatenate([tri, tri], axis=1)  # [128, 256]

    wq_s = (wq * scale).astype(bf16)
    wk_s = wk.astype(bf16)
    wv_s = wv.astype(bf16)
    wo_s = wo.astype(bf16)
    xTb = [np.ascontiguousarray(x[b].T.astype(bf16)) for b in range(B)]

    in_maps = []
    for cix in range(NCORES):
        b = cix // 4
        g = cix % 4                       # head group (4 heads)
        rows = slice(g * CPC, (g + 1) * CPC)
        qblocks, kvblocks = [], []
        for h in range(HPC):
            hr = slice((g * HPC + h) * HD, (g * HPC + h + 1) * HD)
            qblocks.append(wq_s[hr])      # q_h: [128, D]
        for h in range(HPC):
            hr = slice((g * HPC + h) * HD, (g * HPC + h + 1) * HD)
            kvblocks.append(wk_s[hr])
        kvblocks.append(wv_s[rows])       # v all 4 heads: [512, D]
        wqT = np.ascontiguousarray(np.concatenate(qblocks, axis=0).T)
        wkvT = np.ascontiguousarray(np.concatenate(kvblocks, axis=0).T)
        woT = np.ascontiguousarray(wo_s[:, rows].T)  # [512, D]
        in_maps.append({
            "xT": xTb[b],
            "wqT": wqT,
            "wkvT": wkvT,
            "woT": woT,
            "cosT": cosT,
            "sinT": sinT,
            "rotL": rotL,
            "trimask": trimask,
            "ones": np.ones((P, P), dtype=bf16),
        })
    return in_maps


def _get_nc():
    global _nc_cache
    if _nc_cache is None:
        _nc_cache = _build_nc()
    return _nc_cache


def kernel(x, wq, wk, wv, wo, _trace=False):
    global last_exec_time_ns
    nc = _get_nc()
    in_maps = _host_inputs(np.asarray(x, dtype=np.float32),
                           np.asarray(wq, dtype=np.float32),
                           np.asarray(wk, dtype=np.float32),
                           np.asarray(wv, dtype=np.float32),
                           np.asarray(wo, dtype=np.float32))
    res = run_bass_kernel_spmd(nc, in_maps, core_ids=list(range(NCORES)),
                               trace=_trace)
    last_exec_time_ns = res.exec_time_ns
    y = np.zeros((B, S, D), dtype=np.float64)
    for cix in range(NCORES):
        b = cix // 4
        y[b] += res.results[cix]["yT"].T.astype(np.float64)
    return y.astype(np.float32)
